# revision 10
# baseline (speedup 1.0000x reference)
"""ViT-Base + per-sample MoE adapters on 8 TRN2 NeuronCores.

Sharding: data-parallel over batch (4 samples/core, zero collectives).
Device layout: feature-major activations xT[d, t] (d on partitions, 6 chunks
of 128), bf16 matmul operands, fp32 residual. Scores are computed transposed
(sT[j,i]) so the softmax reduction becomes a ones-matmul and no on-chip
transposes are needed anywhere. LN gamma/beta and layer-scale are folded into
weights on the host; the adapter expert gather happens on the host during
sharding (it is per-sample indexing, i.e. data movement, not compute).
"""

import sys

sys.path.insert(0, "/opt/trn_rl_repo")
sys.path.insert(0, "/root/.axon_site/_ro/trn_rl_repo")

from contextlib import ExitStack

import numpy as np
import ml_dtypes
from einops import rearrange

import concourse.bass as bass
import concourse.tile as tile
import concourse.mybir as mybir
from concourse import bacc
from concourse.bass_utils import run_bass_kernel_spmd

F32 = mybir.dt.float32
BF16 = mybir.dt.bfloat16
AF = mybir.ActivationFunctionType
ALU = mybir.AluOpType
BF = ml_dtypes.bfloat16

B, IMG, PP, CIN = 32, 224, 16, 3
D, H, L, A, E, DD, FF = 768, 12, 12, 6, 8, 64, 3072
G = IMG // PP         # 14
N = G * G + 1         # 197
HD = D // H           # 64
NCORES = 8
S = B // NCORES       # 4 samples per core
T = S * N             # 788 tokens per core
DC = D // 128         # 6 chunks
QKC = 12              # q(6) + k(6) feature chunks
FJ = FF // 128        # 24
PC = (CIN * PP * PP) // 128  # 18
NPATCH = G * G        # 196
TB = 2
TBW = T // TB         # 394
EPS = 1e-6

_CACHE = {}


def _f(x):
    return np.asarray(x, np.float32)


def _prep(inputs):
    """Host-side prep: im2col, LN/LS folds, expert gather, bf16 packs."""
    pw = _f(inputs["patch_w"]); pb = _f(inputs["patch_b"])
    cls = _f(inputs["cls_token"]); pos = _f(inputs["pos_embed"])
    l1g = _f(inputs["ln1_g"]); l1b = _f(inputs["ln1_b"])
    qkvw = _f(inputs["qkv_w"]); qkvb = _f(inputs["qkv_b"])
    pjw = _f(inputs["proj_w"]); pjb = _f(inputs["proj_b"])
    ls1 = _f(inputs["ls1"]); ls2 = _f(inputs["ls2"])
    l2g = _f(inputs["ln2_g"]); l2b = _f(inputs["ln2_b"])
    f1w = _f(inputs["fc1_w"]); f1b = _f(inputs["fc1_b"])
    f2w = _f(inputs["fc2_w"]); f2b = _f(inputs["fc2_b"])
    ng = _f(inputs["norm_g"]); nb = _f(inputs["norm_b"])
    adw = _f(inputs["ad_down_w"]); adb = _f(inputs["ad_down_b"])
    auw = _f(inputs["ad_up_w"]); aub = _f(inputs["ad_up_b"])
    eids = np.asarray(inputs["expert_ids"], np.int64)
    imgs = _f(inputs["inputs"])

    shared = {}
    qw = qkvw[:, :, :D]; kw = qkvw[:, :, D:2 * D]; vw = qkvw[:, :, 2 * D:]
    wqk = np.concatenate([qw, kw], axis=2) * l1g[:, :, None]          # [L,768,1536]
    shared["wqk"] = rearrange(wqk, "l (c pc) (j pj) -> l j pc c pj", pc=128, pj=128).astype(BF)
    qkb = np.einsum("ldk,ld->lk", np.concatenate([qw, kw], axis=2), l1b) + qkvb[:, :2 * D]
    shared["qkb"] = rearrange(qkb, "l (j pj) -> l pj j", pj=128).astype(np.float32)

    wv = vw * l1g[:, :, None]
    shared["wv"] = rearrange(wv, "l (c pc) d -> l pc c d", pc=128).astype(BF)
    vb = np.einsum("ldk,ld->lk", vw, l1b) + qkvb[:, 2 * D:]           # [L,768]

    wproj = pjw * ls1[:, None, :]
    shared["wproj"] = rearrange(wproj, "l (c pc) (j pj) -> l j pc c pj", pc=128, pj=128).astype(BF)
    pbe = ls1 * (pjb + np.einsum("ldk,ld->lk", pjw, vb))
    shared["pbe"] = rearrange(pbe, "l (j pj) -> l pj j", pj=128).astype(np.float32)

    wfc1 = f1w * l2g[:, :, None]
    shared["wfc1"] = rearrange(wfc1, "l (c pc) (j pj) -> l j pc c pj", pc=128, pj=128).astype(BF)
    f1be = np.einsum("ldk,ld->lk", f1w, l2b) + f1b
    shared["f1be"] = rearrange(f1be, "l (j pj) -> l pj j", pj=128).astype(np.float32)

    wfc2 = f2w * ls2[:, None, :]
    shared["wfc2"] = rearrange(wfc2, "l (c pc) (j pj) -> l j pc c pj", pc=128, pj=128).astype(BF)
    f2be = ls2 * f2b
    shared["f2be"] = rearrange(f2be, "l (j pj) -> l pj j", pj=128).astype(np.float32)

    wpatch = pw.T  # [2304, 768]
    shared["wpatch"] = rearrange(wpatch, "(c pc) (j pj) -> j pc c pj", pc=128, pj=128).astype(BF)

    posb = pos[0].copy()                  # [197, 768]
    posb[1:] += pb[None, :]
    posb[0] += cls[0, 0]
    shared["posbias"] = rearrange(posb, "t (c pc) -> pc c t", pc=128).astype(np.float32)

    shared["wnorm"] = np.stack([
        rearrange(ng, "(c pc) -> pc c", pc=128),
        rearrange(nb, "(c pc) -> pc c", pc=128)], axis=-1).astype(np.float32)  # [128,6,2]

    flags = dict(
        has_pbe=bool(np.abs(pbe).max() > 0),
        has_f2be=bool(np.abs(f2be).max() > 0),
        has_qkb=bool(np.abs(qkb).max() > 0),
        has_adb=bool(np.abs(adb).max() > 0),
        has_aub=bool(np.abs(aub).max() > 0),
    )

    per_core = []
    for core in range(NCORES):
        sl = slice(core * S, (core + 1) * S)
        im = imgs[sl]
        xp = im.reshape(S, CIN, G, PP, G, PP).transpose(0, 2, 4, 1, 3, 5).reshape(
            S * NPATCH, CIN * PP * PP)
        xpT = rearrange(np.ascontiguousarray(xp.T), "(c pc) t -> c pc t", pc=128).astype(BF)
        eid = eids[sl]
        pc_map = {
            "xpT": xpT,                                              # [18,128,784]
            "adwg": rearrange(adw[:, eid], "a s (c pc) k -> a pc c s k", pc=128).astype(BF),
            "auwg": rearrange(auw[:, eid], "a s k d -> a k s d").astype(BF),   # [A,64,S,768]
            "adbg": rearrange(adb[:, eid], "a s k -> a k s").astype(np.float32),
            "aubg": rearrange(aub[:, eid], "a s (j pj) -> a s pj j", pj=128).astype(np.float32),
        }
        per_core.append(pc_map)
    return shared, per_core, flags


def _build(flags, n_layers=L, dbg=False):
    key = (tuple(sorted(flags.items())), n_layers, dbg)
    if key in _CACHE:
        return _CACHE[key]
    nc = bacc.Bacc("TRN2", target_bir_lowering=False, debug=False, num_devices=NCORES)

    def din(name, shape, dt):
        return nc.dram_tensor(name, list(shape), dt, kind="ExternalInput").ap()

    xpT_d = din("xpT", [PC, 128, S * NPATCH], BF16)
    posb_d = din("posbias", [128, DC, N], F32)
    wqk_d = din("wqk", [L, QKC, 128, DC, 128], BF16)
    qkb_d = din("qkb", [L, 128, QKC], F32)
    wv_d = din("wv", [L, 128, DC, D], BF16)
    wproj_d = din("wproj", [L, DC, 128, DC, 128], BF16)
    pbe_d = din("pbe", [L, 128, DC], F32)
    wfc1_d = din("wfc1", [L, FJ, 128, DC, 128], BF16)
    f1be_d = din("f1be", [L, 128, FJ], F32)
    wfc2_d = din("wfc2", [L, DC, 128, FJ, 128], BF16)
    f2be_d = din("f2be", [L, 128, DC], F32)
    wpatch_d = din("wpatch", [DC, 128, PC, 128], BF16)
    wnorm_d = din("wnorm", [128, DC, 2], F32)
    adwg_d = din("adwg", [A, 128, DC, S, DD], BF16)
    auwg_d = din("auwg", [A, DD, S, D], BF16)
    adbg_d = din("adbg", [A, DD, S], F32)
    aubg_d = din("aubg", [A, S, 128, DC], F32)

    out_d = nc.dram_tensor("out", [S, D], F32, kind="ExternalOutput")
    if dbg:
        xdbg_d = nc.dram_tensor("xdbg", [128, DC, T], F32, kind="ExternalOutput").ap()

    scol = [slice(s * N, (s + 1) * N) for s in range(S)]
    tbcol = [slice(tb * TBW, (tb + 1) * TBW) for tb in range(TB)]
    jts = [(0, 128), (128, N)]

    with tile.TileContext(nc) as tc:
        with ExitStack() as ctx:
            per = ctx.enter_context(tc.tile_pool(name="per", bufs=1))
            wq_p = ctx.enter_context(tc.tile_pool(name="wq", bufs=3))
            wf2_p = ctx.enter_context(tc.tile_pool(name="wf2", bufs=2))
            xp_p = ctx.enter_context(tc.tile_pool(name="xp", bufs=3))
            ad_p = ctx.enter_context(tc.tile_pool(name="ad", bufs=2))
            bia_p = ctx.enter_context(tc.tile_pool(name="bia", bufs=2))
            st_p = ctx.enter_context(tc.tile_pool(name="st", bufs=2))
            exp_p = ctx.enter_context(tc.tile_pool(name="exp", bufs=4))
            lno_p = ctx.enter_context(tc.tile_pool(name="lno", bufs=2))
            sq_p = ctx.enter_context(tc.tile_pool(name="sq", bufs=1))
            xbf_p = ctx.enter_context(tc.tile_pool(name="xbf", bufs=1))
            ps_mm = ctx.enter_context(tc.tile_pool(name="psmm", bufs=2, space="PSUM"))
            ps_at = ctx.enter_context(tc.tile_pool(name="psat", bufs=4, space="PSUM"))
            ps_sm = ctx.enter_context(tc.tile_pool(name="pssm", bufs=2, space="PSUM"))

            x = per.tile([128, DC, T], F32, tag="x")
            qk = per.tile([128, QKC, T], BF16, tag="qk")
            v_tok = per.tile([128, S, 2, D], BF16, tag="vtok")
            attn = per.tile([128, DC, T], BF16, tag="attn")
            hml = per.tile([128, FJ, TBW], BF16, tag="hml")
            wv_t = per.tile([128, DC, D], BF16, tag="wvt")
            posb = per.tile([128, DC, N], F32, tag="posb")
            ones1 = per.tile([128, 1], BF16, tag="ones1")
            wnorm_t = per.tile([128, DC, 2], F32, tag="wnormt")

            nc.vector.memset(ones1[:], 1.0)
            czero = per.tile([128, 1], F32, tag="czero")
            nc.vector.memset(czero[:], 0.0)
            ceps = per.tile([128, 1], F32, tag="ceps")
            nc.vector.memset(ceps[:], EPS)
            nc.const_aps.aps[(F32, 0.0)] = czero[:]
            nc.const_aps.aps[(F32, EPS)] = ceps[:]
            nc.sync.dma_start(out=posb[:], in_=posb_d[:])
            nc.sync.dma_start(out=wnorm_t[:], in_=wnorm_d[:])

            # ======== patch embed ========
            for j in range(DC):
                wpj = wq_p.tile([128, PC, 128], BF16, tag="wq")
                nc.sync.dma_start(out=wpj[:], in_=wpatch_d[j])
                ps0 = ps_mm.tile([128, 2 * NPATCH], F32, tag="mm")
                ps1 = ps_mm.tile([128, 2 * NPATCH], F32, tag="mm")
                for c in range(PC):
                    xpc = xp_p.tile([128, S * NPATCH], BF16, tag="xp")
                    nc.sync.dma_start(out=xpc[:], in_=xpT_d[c])
                    nc.tensor.matmul(ps0[:], wpj[:, c, :], xpc[:, :2 * NPATCH],
                                     start=(c == 0), stop=(c == PC - 1))
                    nc.tensor.matmul(ps1[:], wpj[:, c, :], xpc[:, 2 * NPATCH:],
                                     start=(c == 0), stop=(c == PC - 1))
                for s in range(S):
                    ps, si = (ps0, s) if s < 2 else (ps1, s - 2)
                    nc.vector.tensor_tensor(
                        x[:, j, s * N + 1:(s + 1) * N],
                        ps[:, si * NPATCH:(si + 1) * NPATCH],
                        posb[:, j, 1:N], ALU.add)
                for s in range(S):
                    nc.vector.tensor_copy(x[:, j, s * N:s * N + 1], posb[:, j, 0:1])

            def layernorm_stats(src_bf, sqt):
                """src_bf bf16 [128, DC, T] -> per-tb broadcast ab tiles [128,2,TBW]."""
                abbs = []
                for tb in range(TB):
                    for c in range(DC):
                        nc.scalar.activation(sqt[:, c, :], src_bf[:, c, tbcol[tb]], AF.Square)
                    sm_x = ps_sm.tile([1, TBW], F32, tag="sm")
                    sm_q = ps_sm.tile([1, TBW], F32, tag="sm")
                    for c in range(DC):
                        nc.tensor.matmul(sm_x[:], ones1[:], src_bf[:, c, tbcol[tb]],
                                         start=(c == 0), stop=(c == DC - 1))
                    for c in range(DC):
                        nc.tensor.matmul(sm_q[:], ones1[:], sqt[:, c, :],
                                         start=(c == 0), stop=(c == DC - 1))
                    mean = st_p.tile([1, TBW], F32, tag="mean")
                    nc.vector.tensor_scalar_mul(mean[:], sm_x[:], 1.0 / D)
                    var = st_p.tile([1, TBW], F32, tag="var")
                    nc.vector.tensor_scalar_mul(var[:], sm_q[:], 1.0 / D)
                    msq = st_p.tile([1, TBW], F32, tag="msq")
                    nc.vector.tensor_tensor(msq[:], mean[:], mean[:], ALU.mult)
                    nc.vector.tensor_tensor(var[:], var[:], msq[:], ALU.subtract)
                    r = st_p.tile([1, TBW], F32, tag="r")
                    nc.scalar.activation(r[:], var[:], AF.Ln, bias=EPS)
                    nc.scalar.activation(r[:], r[:], AF.Exp, scale=-0.5)
                    ab = st_p.tile([1, 2, TBW], BF16, tag="ab")
                    nc.vector.tensor_copy(ab[:, 0, :], r[:])
                    mr = st_p.tile([1, TBW], F32, tag="mr")
                    nc.vector.tensor_tensor(mr[:], mean[:], r[:], ALU.mult)
                    nc.vector.tensor_scalar_mul(ab[:, 1, :], mr[:], -1.0)
                    abb = st_p.tile([128, 2, TBW], BF16, tag="abb")
                    nc.gpsimd.partition_broadcast(abb[:], ab[0:1, :, :])
                    abbs.append(abb)
                return abbs

            def ln_apply(abbs, src_bf, dst):
                for tb in range(TB):
                    for c in range(DC):
                        nc.vector.tensor_tensor(dst[:, c, tbcol[tb]], src_bf[:, c, tbcol[tb]],
                                                abbs[tb][:, 0, :], ALU.mult)
                        nc.vector.tensor_tensor(dst[:, c, tbcol[tb]], dst[:, c, tbcol[tb]],
                                                abbs[tb][:, 1, :], ALU.add)

            # ======== transformer layers ========
            for l in range(n_layers):
                # ---- LN1 ----
                x_bf = xbf_p.tile([128, DC, T], BF16, tag="xbf")
                for tb in range(TB):
                    for c in range(DC):
                        nc.vector.tensor_copy(x_bf[:, c, tbcol[tb]], x[:, c, tbcol[tb]])
                sqt = sq_p.tile([128, DC, TBW], BF16, tag="sq")
                abbs = layernorm_stats(x_bf, sqt)
                ln1 = lno_p.tile([128, DC, T], BF16, tag="lno")
                ln_apply(abbs, x_bf, ln1)

                # ---- QK ----
                if flags["has_qkb"]:
                    qkb_t = bia_p.tile([128, QKC], F32, tag="qkb")
                    nc.sync.dma_start(out=qkb_t[:], in_=qkb_d[l])
                for j in range(QKC):
                    wj = wq_p.tile([128, DC, 128], BF16, tag="wq")
                    nc.sync.dma_start(out=wj[:], in_=wqk_d[l, j])
                    for tb in range(TB):
                        ps = ps_mm.tile([128, TBW], F32, tag="mm")
                        for c in range(DC):
                            nc.tensor.matmul(ps[:], wj[:, c, :], ln1[:, c, tbcol[tb]],
                                             start=(c == 0), stop=(c == DC - 1))
                        if flags["has_qkb"]:
                            nc.scalar.activation(qk[:, j, tbcol[tb]], ps[:], AF.Identity,
                                                 bias=qkb_t[:, j:j + 1])
                        else:
                            nc.scalar.copy(qk[:, j, tbcol[tb]], ps[:])

                # ---- V (token-major) ----
                nc.sync.dma_start(out=wv_t[:], in_=wv_d[l])
                for s in range(S):
                    for jt, (j0, j1) in enumerate(jts):
                        tn = j1 - j0
                        for nh in range(2):
                            ps = ps_mm.tile([128, 384], F32, tag="mm")
                            for c in range(DC):
                                nc.tensor.matmul(
                                    ps[:tn, :], ln1[:, c, s * N + j0:s * N + j1],
                                    wv_t[:, c, nh * 384:(nh + 1) * 384],
                                    start=(c == 0), stop=(c == DC - 1))
                            nc.vector.tensor_copy(v_tok[:tn, s, jt, nh * 384:(nh + 1) * 384],
                                                  ps[:tn, :])

                # ---- attention ----
                for s in range(S):
                    for a in range(DC):       # head pair a: heads 2a / 2a+1
                        sT_e = ps_at.tile([128, 2, N], F32, tag="at")
                        sT_o = ps_at.tile([128, 2, N], F32, tag="at")
                        for jt, (j0, j1) in enumerate(jts):
                            tn = j1 - j0
                            nc.tensor.matmul(
                                sT_e[:tn, jt, :], qk[0:64, DC + a, scol[s]][:, j0:j1],
                                qk[0:64, a, scol[s]], start=True, stop=True,
                                tile_position=(0, 0))
                            nc.tensor.matmul(
                                sT_o[:tn, jt, :], qk[64:128, DC + a, scol[s]][:, j0:j1],
                                qk[64:128, a, scol[s]], start=True, stop=True,
                                tile_position=(64, 0))
                        expe = exp_p.tile([128, 2, N], BF16, tag="exp")
                        expo = exp_p.tile([128, 2, N], BF16, tag="exp")
                        for ps_h, ex in ((sT_e, expe), (sT_o, expo)):
                            nc.scalar.activation(ex[:, 0, :], ps_h[:, 0, :], AF.Exp, scale=0.125)
                            nc.scalar.activation(ex[:69, 1, :], ps_h[:69, 1, :], AF.Exp,
                                                 scale=0.125)
                        cs_e = ps_sm.tile([1, N], F32, tag="sm")
                        cs_o = ps_sm.tile([1, N], F32, tag="sm")
                        nc.tensor.matmul(cs_e[:], ones1[:], expe[:, 0, :], start=True, stop=False)
                        nc.tensor.matmul(cs_e[:], ones1[:69, :], expe[:69, 1, :], start=False,
                                         stop=True)
                        nc.tensor.matmul(cs_o[:], ones1[:], expo[:, 0, :], start=True, stop=False)
                        nc.tensor.matmul(cs_o[:], ones1[:69, :], expo[:69, 1, :], start=False,
                                         stop=True)
                        rec_e = st_p.tile([1, N], F32, tag="rec")
                        rec_o = st_p.tile([1, N], F32, tag="rec")
                        nc.vector.reciprocal(rec_e[:], cs_e[:])
                        nc.vector.reciprocal(rec_o[:], cs_o[:])
                        rcb_e = st_p.tile([128, N], F32, tag="rcb")
                        rcb_o = st_p.tile([128, N], F32, tag="rcb")
                        nc.gpsimd.partition_broadcast(rcb_e[:], rec_e[:])
                        nc.gpsimd.partition_broadcast(rcb_o[:], rec_o[:])
                        psA = ps_at.tile([128, N], F32, tag="at")
                        psB = ps_at.tile([128, N], F32, tag="at")
                        dlo = a * 128
                        nc.tensor.matmul(psA[:], v_tok[:, s, 0, dlo:dlo + 128], expe[:, 0, :],
                                         start=True, stop=False)
                        nc.tensor.matmul(psA[:], v_tok[:69, s, 1, dlo:dlo + 128],
                                         expe[:69, 1, :], start=False, stop=True)
                        nc.tensor.matmul(psB[:], v_tok[:, s, 0, dlo:dlo + 128], expo[:, 0, :],
                                         start=True, stop=False)
                        nc.tensor.matmul(psB[:], v_tok[:69, s, 1, dlo:dlo + 128],
                                         expo[:69, 1, :], start=False, stop=True)
                        nc.vector.tensor_tensor(attn[0:64, a, scol[s]], psA[0:64, :],
                                                rcb_e[0:64, :], ALU.mult)
                        nc.vector.tensor_tensor(attn[64:128, a, scol[s]], psB[64:128, :],
                                                rcb_o[64:128, :], ALU.mult)

                # ---- proj + residual ----
                if flags["has_pbe"]:
                    pbe_t = bia_p.tile([128, DC], F32, tag="pbe")
                    nc.sync.dma_start(out=pbe_t[:], in_=pbe_d[l])
                for j in range(DC):
                    wj = wq_p.tile([128, DC, 128], BF16, tag="wq")
                    nc.sync.dma_start(out=wj[:], in_=wproj_d[l, j])
                    for tb in range(TB):
                        ps = ps_mm.tile([128, TBW], F32, tag="mm")
                        for c in range(DC):
                            nc.tensor.matmul(ps[:], wj[:, c, :], attn[:, c, tbcol[tb]],
                                             start=(c == 0), stop=(c == DC - 1))
                        if flags["has_pbe"]:
                            nc.vector.scalar_tensor_tensor(
                                x[:, j, tbcol[tb]], ps[:], pbe_t[:, j:j + 1],
                                x[:, j, tbcol[tb]], ALU.add, ALU.add)
                        else:
                            nc.vector.tensor_tensor(x[:, j, tbcol[tb]], ps[:],
                                                    x[:, j, tbcol[tb]], ALU.add)

                # ---- cast post-attn x, LN2 stats ----
                x_bf2 = xbf_p.tile([128, DC, T], BF16, tag="xbf")
                for tb in range(TB):
                    for c in range(DC):
                        nc.vector.tensor_copy(x_bf2[:, c, tbcol[tb]], x[:, c, tbcol[tb]])
                sqt2 = sq_p.tile([128, DC, TBW], BF16, tag="sq")
                abbs2 = layernorm_stats(x_bf2, sqt2)
                ln2 = lno_p.tile([128, DC, T], BF16, tag="lno")
                ln_apply(abbs2, x_bf2, ln2)

                # ---- adapter ----
                if l < A:
                    adw_t = ad_p.tile([128, DC, S, DD], BF16, tag="adw")
                    auw_t = ad_p.tile([DD, S, D], BF16, tag="auw")
                    nc.sync.dma_start(out=adw_t[:], in_=adwg_d[l])
                    nc.sync.dma_start(out=auw_t[:], in_=auwg_d[l])
                    if flags["has_adb"]:
                        adbg_t = bia_p.tile([DD, S], F32, tag="adb")
                        nc.sync.dma_start(out=adbg_t[:], in_=adbg_d[l])
                    if flags["has_aub"]:
                        aubg_t = bia_p.tile([S, 128, DC], F32, tag="aub")
                        nc.sync.dma_start(out=aubg_t[:], in_=aubg_d[l])
                    for s in range(S):
                        psh = ps_mm.tile([DD, N], F32, tag="mm")
                        for c in range(DC):
                            nc.tensor.matmul(psh[:], adw_t[:, c, s, :], x_bf2[:, c, scol[s]],
                                             start=(c == 0), stop=(c == DC - 1))
                        hp = st_p.tile([DD, N], BF16, tag="hp")
                        if flags["has_adb"]:
                            nc.scalar.activation(hp[:], psh[:], AF.Gelu,
                                                 bias=adbg_t[:, s:s + 1])
                        else:
                            nc.scalar.activation(hp[:], psh[:], AF.Gelu)
                        for j in range(DC):
                            psu = ps_mm.tile([128, N], F32, tag="mm")
                            nc.tensor.matmul(psu[:], auw_t[:, s, j * 128:(j + 1) * 128],
                                             hp[:], start=True, stop=True)
                            if flags["has_aub"]:
                                nc.vector.scalar_tensor_tensor(
                                    x[:, j, scol[s]], psu[:], aubg_t[s, :, j:j + 1],
                                    x[:, j, scol[s]], ALU.add, ALU.add)
                            else:
                                nc.vector.tensor_tensor(x[:, j, scol[s]], psu[:],
                                                        x[:, j, scol[s]], ALU.add)

                # ---- MLP ----
                f1be_t = bia_p.tile([128, FJ], F32, tag="f1b")
                nc.sync.dma_start(out=f1be_t[:], in_=f1be_d[l])
                if flags["has_f2be"]:
                    f2be_t = bia_p.tile([128, DC], F32, tag="f2b")
                    nc.sync.dma_start(out=f2be_t[:], in_=f2be_d[l])
                for tb in range(TB):
                    for j in range(FJ):
                        wj = wq_p.tile([128, DC, 128], BF16, tag="wq")
                        nc.sync.dma_start(out=wj[:], in_=wfc1_d[l, j])
                        ps = ps_mm.tile([128, TBW], F32, tag="mm")
                        for c in range(DC):
                            nc.tensor.matmul(ps[:], wj[:, c, :], ln2[:, c, tbcol[tb]],
                                             start=(c == 0), stop=(c == DC - 1))
                        nc.scalar.activation(hml[:, j, :], ps[:], AF.Gelu,
                                             bias=f1be_t[:, j:j + 1])
                    for j in range(DC):
                        w2j = wf2_p.tile([128, FJ, 128], BF16, tag="wf2")
                        nc.sync.dma_start(out=w2j[:], in_=wfc2_d[l, j])
                        ps = ps_mm.tile([128, TBW], F32, tag="mm")
                        for c in range(FJ):
                            nc.tensor.matmul(ps[:], w2j[:, c, :], hml[:, c, :],
                                             start=(c == 0), stop=(c == FJ - 1))
                        if flags["has_f2be"]:
                            nc.vector.scalar_tensor_tensor(
                                x[:, j, tbcol[tb]], ps[:], f2be_t[:, j:j + 1],
                                x[:, j, tbcol[tb]], ALU.add, ALU.add)
                        else:
                            nc.vector.tensor_tensor(x[:, j, tbcol[tb]], ps[:],
                                                    x[:, j, tbcol[tb]], ALU.add)

            if dbg:
                for c in range(DC):
                    nc.sync.dma_start(out=xdbg_d[:, c, :], in_=x[:, c, :])

            # ======== final LN on CLS columns + output ========
            xc = st_p.tile([128, DC, S], F32, tag="xc")
            for c in range(DC):
                nc.vector.tensor_copy(xc[:, c, :], x[:, c, ::N])
            xcb = st_p.tile([128, DC, S], BF16, tag="xcb")
            nc.vector.tensor_copy(xcb[:], xc[:])
            sqc = st_p.tile([128, DC, S], BF16, tag="sqc")
            nc.scalar.activation(sqc[:], xcb[:], AF.Square)
            fs_x = ps_sm.tile([1, S], F32, tag="sm")
            fs_q = ps_sm.tile([1, S], F32, tag="sm")
            for c in range(DC):
                nc.tensor.matmul(fs_x[:], ones1[:], xcb[:, c, :], start=(c == 0),
                                 stop=(c == DC - 1))
            for c in range(DC):
                nc.tensor.matmul(fs_q[:], ones1[:], sqc[:, c, :], start=(c == 0),
                                 stop=(c == DC - 1))
            fmean = st_p.tile([1, S], F32, tag="fmean")
            nc.vector.tensor_scalar_mul(fmean[:], fs_x[:], 1.0 / D)
            var = st_p.tile([1, S], F32, tag="fvar")
            nc.vector.tensor_scalar_mul(var[:], fs_q[:], 1.0 / D)
            fmsq = st_p.tile([1, S], F32, tag="fmsq")
            nc.vector.tensor_tensor(fmsq[:], fmean[:], fmean[:], ALU.mult)
            nc.vector.tensor_tensor(var[:], var[:], fmsq[:], ALU.subtract)
            r = st_p.tile([1, S], F32, tag="fr")
            nc.scalar.activation(r[:], var[:], AF.Ln, bias=EPS)
            nc.scalar.activation(r[:], r[:], AF.Exp, scale=-0.5)
            rb = st_p.tile([128, S], F32, tag="frb")
            nc.gpsimd.partition_broadcast(rb[:], r[:])
            mb = st_p.tile([128, S], F32, tag="fmb")
            nc.gpsimd.partition_broadcast(mb[:], fmean[:])
            on = st_p.tile([128, DC, S], F32, tag="on")
            for c in range(DC):
                nc.vector.tensor_tensor(on[:, c, :], xc[:, c, :], mb[:], ALU.subtract)
                nc.vector.tensor_tensor(on[:, c, :], on[:, c, :], rb[:], ALU.mult)
                nc.vector.tensor_scalar(on[:, c, :], on[:, c, :],
                                        wnorm_t[:, c, 0:1], wnorm_t[:, c, 1:2],
                                        ALU.mult, ALU.add)
            for c in range(DC):
                dst = bass.AP(tensor=out_d, offset=c * 128, ap=[[1, 128], [D, S]])
                nc.sync.dma_start(out=dst, in_=on[:, c, :])

    nc.compile()
    _CACHE[key] = nc
    return nc


def kernel(_n_layers=L, _dbg=False, **inputs):
    shared, per_core, flags = _prep(inputs)
    nc = _build(flags, n_layers=_n_layers, dbg=_dbg)
    in_maps = []
    for core in range(NCORES):
        m = dict(shared)
        m.update(per_core[core])
        in_maps.append(m)
    res = run_bass_kernel_spmd(nc, in_maps, core_ids=list(range(NCORES)))
    out = np.concatenate([res.results[i]["out"] for i in range(NCORES)], axis=0)
    if _dbg:
        xd = [res.results[i]["xdbg"] for i in range(NCORES)]
        return out.astype(np.float32), xd
    return out.astype(np.float32)
